# revision 1
# baseline (speedup 1.0000x reference)
"""Trainium2 kernel for nn_AlephPracticalEngine (sparse_attention).

The reference computes out = fhh(fhh(x) * w + gather-einsum) / DIM, which is
linear in x: out = x @ W^T with W = M @ (diag(w) + S) @ M / DIM, where M is
the fhh butterfly matrix and S is scatter-added from the engram tables.
The (x-independent) weight fold runs on host; the device does one dense
[2048,4096] x [4096,4096] bf16 matmul, sharded 2 (batch) x 4 (features)
across 8 NeuronCores.
"""

import numpy as np
import ml_dtypes

DIM = 4096
BATCH = 2048
N_CORES = 8
ROW_GROUPS = 2  # batch groups
COL_GROUPS = 4  # output-feature groups
M_C = BATCH // ROW_GROUPS  # 1024 batch rows per core
N_C = DIM // COL_GROUPS  # 1024 output features per core

_compiled_nc = None


def _fhh_np(x):
    """numpy mirror of the reference fhh butterfly (last axis)."""
    n = x.shape[-1]
    if n == 1:
        return x
    half = n // 2
    left, right = x[..., :half], x[..., half:]
    h_sum = left + right
    h_diff = left - right
    h_diff2 = h_diff + np.roll(h_diff, shift=1, axis=-1)
    return np.concatenate([_fhh_np(h_sum), _fhh_np(h_diff2)], axis=-1)


def _build_Wt(spectral_weights, engram_values, engram_indices, dtype=np.float32):
    """W^T [k, d] for out = x @ W^T, W = M (diag(w) + S) M / DIM."""
    w = np.asarray(spectral_weights, dtype)
    val = np.asarray(engram_values, dtype)
    idx = np.asarray(engram_indices).astype(np.int64)
    D, K = val.shape
    # M^T = fhh(I):  fhh(e_j)[i] = M[i, j]
    M = np.ascontiguousarray(_fhh_np(np.eye(D, dtype=dtype)).T)
    AM = w[:, None] * M
    for k in range(K):
        AM += val[:, k][:, None] * M[idx[:, k], :]
    # W = fhh applied along axis 0 of AM, / D;  W^T = fhh(AM^T) / D
    Wt = _fhh_np(np.ascontiguousarray(AM.T)) / D
    return np.ascontiguousarray(Wt)


def _get_compiled():
    global _compiled_nc
    if _compiled_nc is None:
        import concourse.mybir as mybir
        import concourse.tile as tile
        from concourse import bacc
        from concourse.kernels.tile_matmul import matmul_tile_kernel

        nc = bacc.Bacc(
            "TRN2",
            target_bir_lowering=False,
            debug=False,
            num_devices=N_CORES,
        )
        xT = nc.dram_tensor("xT", [DIM, M_C], mybir.dt.bfloat16, kind="ExternalInput")
        wT = nc.dram_tensor("wT", [DIM, N_C], mybir.dt.bfloat16, kind="ExternalInput")
        out = nc.dram_tensor("out", [M_C, N_C], mybir.dt.float32, kind="ExternalOutput")
        with tile.TileContext(nc) as tc:
            matmul_tile_kernel(tc, xT.ap(), wT.ap(), out.ap())
        nc.compile()
        _compiled_nc = nc
    return _compiled_nc


def _prepare_in_maps(inputs):
    x = np.asarray(inputs["x"], np.float32)
    Wt = _build_Wt(
        inputs["spectral_weights"], inputs["engram_values"], inputs["engram_indices"]
    )
    xb = x.astype(ml_dtypes.bfloat16)
    Wtb = Wt.astype(ml_dtypes.bfloat16)
    in_maps = []
    for c in range(N_CORES):
        r, f = divmod(c, COL_GROUPS)
        in_maps.append(
            {
                "xT": np.ascontiguousarray(xb[r * M_C : (r + 1) * M_C, :].T),
                "wT": np.ascontiguousarray(Wtb[:, f * N_C : (f + 1) * N_C]),
            }
        )
    return in_maps


def _run(in_maps, trace=False):
    from concourse.bass_utils import run_bass_kernel_spmd

    nc = _get_compiled()
    return run_bass_kernel_spmd(
        nc, in_maps, core_ids=list(range(N_CORES)), trace=trace
    )


def _assemble(results):
    out = np.empty((BATCH, DIM), np.float32)
    for c in range(N_CORES):
        r, f = divmod(c, COL_GROUPS)
        out[r * M_C : (r + 1) * M_C, f * N_C : (f + 1) * N_C] = results[c]["out"]
    return out


def kernel(**inputs):
    res = _run(_prepare_in_maps(inputs))
    return _assemble(res.results)
